# revision 13
# baseline (speedup 1.0000x reference)
"""Trainium2 Bass kernel for CrossAttention_Light.

Problem: B=4, L=Lc=4096, D=256, H=4 heads of hd=64.
  q  = x @ Wq + bq                      -> [B,H,L,64]
  k,v = context @ Wkv + bkv             -> [B,H,Lc,64]
  P  = softmax(q k^T / 8)               -> [B,H,L,Lc]
  attn_map = mean_h sum_k P == 1.0 (exactly, softmax rows sum to 1)
  out = (P v) concat-heads @ Wp + bp + x

Sharding: 8 cores = 4 batches x 2 query-halves (2048 rows each). Each core
computes its full output rows end-to-end; no cross-core communication.

Math simplifications (exact):
  - k-bias drops out of softmax (per-row shift invariance).
  - v-bias: P @ (V+bv) = P@V + bv (rows of P sum to 1) -> folded into the
    projection bias: bp_eff = bv @ Wp + bp.
  - softmax without max-subtraction: scores = q.k/8 with |score| < ~1 for
    this weight scale (0.02), exp() is numerically safe.

Device algorithm per core (flash-style, scores never touch HBM):
  - all matmul operands bf16 (f32 PSUM accumulation); errors average over
    the 4096-term attention sums and the f32 residual dominates the output.
  - S^T layout: S^T[k,q] blocks from lhsT=K^T[64,128], rhs=Q^T[64,512].
    Head pairs occupy partition halves (rows 0-63 / 64-127) so the two
    S matmuls (K=64 each) run concurrently in the PE array.
  - exp on ACT (scale=1/8 folded in), one [128,1024] instr per k-block
    covering both heads.
  - AV: lhsT=[V|1] [128,65], rhs=expS^T [128,512] accumulating
    O^T[65,512] in PSUM; row 64 = softmax denominators.
  - normalize: reciprocal (DVE) + PE rank-1 broadcast + DVE multiply.
  - projection: per 128-row block, 4 head matmuls + rank-1 bias matmul,
    residual add (DVE), DMA out.
"""

import os
import sys

for _p in ("/opt/trn_rl_repo",):
    if _p not in sys.path:
        sys.path.append(_p)

import ml_dtypes
import numpy as np
from contextlib import ExitStack

NP_BF16 = ml_dtypes.bfloat16

import concourse.bass as bass
import concourse.mybir as mybir
import concourse.tile as tile
from concourse import bacc
from concourse.bass_utils import run_bass_kernel_spmd

F32 = mybir.dt.float32
BF16 = mybir.dt.bfloat16
AF = mybir.ActivationFunctionType

B, L, D, LC, H, HD = 4, 4096, 256, 4096, 4, 64
N_CORES = 8
LQ = L // 2          # q rows per core
P = 128
SB = 512             # q superblock
SCALE = HD ** -0.5


def build_program(lq=LQ, lc=LC, n_cores=N_CORES):
    """Build the SPMD Bass program. Returns nc."""
    nc = bacc.Bacc(
        "TRN2", target_bir_lowering=False, debug=False, num_devices=n_cores
    )
    nbq = lq // P        # q blocks
    nbc = lc // P        # k blocks
    nsb = lq // SB       # q superblocks
    dc = D // P          # contraction chunks (2)
    hp_n = H // 2        # head pairs

    def din(name, shape, dt=F32):
        return nc.dram_tensor(name, shape, dt, kind="ExternalInput").ap()

    x_d = din("xnat", [P, nbq * D])          # x rows, partition-packed
    xt_d = din("xt", [P, dc * lq], BF16)     # x^T chunks
    ct_d = din("ctxt", [P, dc * lc], BF16)   # context^T chunks
    wq_d = din("wq", [P, dc * D], BF16)      # Wq row-chunked
    wk_d = din("wk", [P, dc * D], BF16)
    wv_d = din("wv", [P, dc * D], BF16)
    wp_d = din("wp4", [HD, H * D], BF16)     # Wp head-major rows
    bq_d = din("bqp", [P, hp_n])             # q bias packed (t d) hp
    bp_d = din("bpe", [1, D], BF16)          # effective out bias
    out_d = nc.dram_tensor("out", [P, nbq * D], F32, kind="ExternalOutput").ap()

    with tile.TileContext(nc) as tc, ExitStack() as ctx:
        _kernel(ctx, tc, x_d, xt_d, ct_d, wq_d, wk_d, wv_d, wp_d, bq_d, bp_d,
                out_d, lq, lc)
    nc.compile()
    return nc


def _kernel(ctx, tc, x_d, xt_d, ct_d, wq_d, wk_d, wv_d, wp_d, bq_d, bp_d,
            out_d, lq, lc):
    nc = tc.nc
    nbq = lq // P
    nbc = lc // P
    nsb = lq // SB
    dc = D // P
    hp_n = H // 2

    persist = ctx.enter_context(tc.tile_pool(name="persist", bufs=1))

    # ---- persistent SBUF tiles ----
    x_sb = persist.tile([P, nbq, D], F32, tag="x")
    qT = [persist.tile([P, lq], BF16, tag=f"qT{i}", name=f"qT{i}")
          for i in range(hp_n)]
    kT = [persist.tile([P, lc], BF16, tag=f"kT{i}", name=f"kT{i}")
          for i in range(hp_n)]
    vones = persist.tile([P, nbc, H, HD + 1], BF16, tag="vones")
    nT = persist.tile([HD, H, lq], BF16, tag="nT")
    wq_sb = persist.tile([P, dc, D], BF16, tag="wq")
    wk_sb = persist.tile([P, dc, D], BF16, tag="wk")
    wv_sb = persist.tile([P, dc, D], BF16, tag="wv")
    wp_sb = persist.tile([HD, H, D], BF16, tag="wp")
    bq_sb = persist.tile([P, hp_n], F32, tag="bq")
    bp_sb = persist.tile([1, D], BF16, tag="bp")
    ones1 = persist.tile([1, P], BF16, tag="ones")
    ones1f = persist.tile([1, P], F32, tag="onesf")

    nc.sync.dma_start(x_sb[:].rearrange("p n d -> p (n d)"), x_d[:])
    nc.sync.dma_start(wq_sb[:].rearrange("p c d -> p (c d)"), wq_d[:])
    nc.sync.dma_start(wk_sb[:].rearrange("p c d -> p (c d)"), wk_d[:])
    nc.sync.dma_start(wv_sb[:].rearrange("p c d -> p (c d)"), wv_d[:])
    nc.sync.dma_start(wp_sb[:].rearrange("p h d -> p (h d)"), wp_d[:])
    nc.sync.dma_start(bq_sb[:], bq_d[:])
    nc.sync.dma_start(bp_sb[:], bp_d[:])
    nc.vector.memset(ones1[:], 1.0)
    nc.vector.memset(ones1f[:], 1.0)
    nc.vector.memset(vones[:, :, :, HD : HD + 1], 1.0)

    # ---- phase A: projections. Order K0, Q0 (unblock attention asap),
    # then V (first AV needs it ~30us in), then K1, Q1 (overlap attention).
    pa_in = ctx.enter_context(tc.tile_pool(name="phaseA_in", bufs=1))
    pq_pool = ctx.enter_context(tc.tile_pool(name="pq", bufs=2, space="PSUM"))
    xt_sb = pa_in.tile([P, dc, lq], BF16, tag="xt")
    ct_sb = pa_in.tile([P, dc, lc], BF16, tag="ct")
    nc.sync.dma_start(ct_sb[:].rearrange("p c q -> p (c q)"), ct_d[:])
    nc.sync.dma_start(xt_sb[:].rearrange("p c q -> p (c q)"), xt_d[:])

    def proj_qk(hp, w_sb, src, src_len, dst, bias):
        # dst[t*64+(0:64), nb*512:+512] = head (2hp+t) projection
        for nb in range(src_len // SB):
            pq = pq_pool.tile([P, SB], F32, tag="pq")
            for t in range(2):
                h = 2 * hp + t
                tp = (0, 64) if t == 1 else None
                for c in range(dc):
                    nc.tensor.matmul(
                        pq[t * HD : (t + 1) * HD, :],
                        w_sb[:, c, h * HD : (h + 1) * HD],
                        src[:, c, nb * SB : (nb + 1) * SB],
                        start=(c == 0), stop=(c == dc - 1),
                        tile_position=tp,
                    )
            sl = dst[:, nb * SB : (nb + 1) * SB]
            if bias is not None:
                nc.vector.tensor_scalar_add(sl, pq[:], bias)
            else:
                nc.vector.tensor_copy(sl, pq[:])

    def proj_v():
        for kb in range(nbc):
            pv = pq_pool.tile([P, D], F32, tag="pq", name="pv")
            for c in range(dc):
                nc.tensor.matmul(
                    pv[:],
                    ct_sb[:, c, kb * P : (kb + 1) * P],
                    wv_sb[:, c, :],
                    start=(c == 0), stop=(c == dc - 1),
                )
            nc.vector.tensor_copy(
                vones[:, kb, :, 0:HD],
                pv[:].rearrange("p (h e) -> p h e", e=HD),
            )

    proj_qk(0, wk_sb, ct_sb, lc, kT[0], None)
    proj_qk(0, wq_sb, xt_sb, lq, qT[0], bq_sb[:, 0:1])
    proj_v()
    proj_qk(1, wk_sb, ct_sb, lc, kT[1], None)
    proj_qk(1, wq_sb, xt_sb, lq, qT[1], bq_sb[:, 1:2])

    # ---- phase B: attention (sb-major so projection can follow each sb),
    # normalize moved off the PSUM critical path via an SBUF copy.
    ps_pool = ctx.enter_context(tc.tile_pool(name="ps", bufs=2, space="PSUM"))
    po_pool = ctx.enter_context(tc.tile_pool(name="po", bufs=2, space="PSUM"))
    es_pool = ctx.enter_context(tc.tile_pool(name="es", bufs=3))
    u_pool = ctx.enter_context(tc.tile_pool(name="u", bufs=4))
    rc_pool = ctx.enter_context(tc.tile_pool(name="rc", bufs=2))
    os_pool = ctx.enter_context(tc.tile_pool(name="os", bufs=3))

    for sb in range(nsb):
        for hp in range(hp_n):
            po = [po_pool.tile([HD + 1, SB], F32, tag="po", name=f"po{t}")
                  for t in range(2)]
            for kb in range(nbc):
                ps = ps_pool.tile([P, 2 * SB], F32, tag="ps")
                for t in range(2):
                    nc.tensor.matmul(
                        ps[:, t * SB : (t + 1) * SB],
                        kT[hp][t * HD : (t + 1) * HD, kb * P : (kb + 1) * P],
                        qT[hp][t * HD : (t + 1) * HD, sb * SB : (sb + 1) * SB],
                        start=True, stop=True,
                    )
                es = es_pool.tile([P, 2 * SB], BF16, tag="es")
                nc.scalar.activation(es[:], ps[:], AF.Exp, scale=SCALE)
                for t in range(2):
                    nc.tensor.matmul(
                        po[t],
                        vones[:, kb, 2 * hp + t, :],
                        es[:, t * SB : (t + 1) * SB],
                        start=(kb == 0), stop=(kb == nbc - 1),
                    )
            for t in range(2):
                h = 2 * hp + t
                # quick PSUM->SBUF copy releases po for the next (sb,hp)
                u = u_pool.tile([HD + 1, SB], F32, tag="u", name=f"u{t}")
                nc.vector.tensor_copy(u[:], po[t])
                rc = rc_pool.tile([1, SB], F32, tag="rc")
                if os.environ.get("CK_RECIP", "fast") == "fast":
                    nc.vector.reciprocal_approx_fast(rc[:], u[HD : HD + 1, :])
                else:
                    nc.vector.reciprocal(rc[:], u[HD : HD + 1, :])
                pb = pq_pool.tile([HD, SB], F32, tag="pq", name="pb")
                nc.tensor.matmul(pb[:], ones1f[0:1, 0:HD], rc[:],
                                 start=True, stop=True)
                nc.vector.tensor_mul(
                    nT[:, h, sb * SB : (sb + 1) * SB], u[0:HD, :], pb[:],
                )
        # ---- projection + residual for this superblock's 4 row-blocks
        for j in range(sb * (SB // P), (sb + 1) * (SB // P)):
            pp = pq_pool.tile([P, D], F32, tag="pq", name="pp")
            for h in range(H):
                nc.tensor.matmul(
                    pp[:],
                    nT[:, h, j * P : (j + 1) * P],
                    wp_sb[:, h, :],
                    start=(h == 0), stop=False,
                )
            nc.tensor.matmul(pp[:], ones1[0:1, :], bp_sb[:],
                             start=False, stop=True)
            ost = os_pool.tile([P, D], F32, tag="os", name="ost")
            nc.vector.tensor_add(ost[:], pp[:], x_sb[:, j, :])
            nc.sync.dma_start(out_d[:, j * D : (j + 1) * D], ost[:])


def _pack_rows(a, nb):
    """[nb*128, d] -> [128, nb*d] partition-packed layout."""
    n, d = a.shape
    return np.ascontiguousarray(
        a.reshape(nb, P, d).transpose(1, 0, 2).reshape(P, nb * d)
    )


def _chunk_rows(w):
    """[256, d] -> [128, 2*d]: row chunks side by side."""
    r, d = w.shape
    c = r // P
    return np.ascontiguousarray(
        w.reshape(c, P, d).transpose(1, 0, 2).reshape(P, c * d)
    )


def make_in_maps(x, context, Wq, bq, Wkv, bkv, Wp, bp, lq=LQ, lc=LC,
                 n_cores=N_CORES):
    x = np.asarray(x, np.float32)
    context = np.asarray(context, np.float32)
    Wq = np.asarray(Wq, np.float32)
    bq = np.asarray(bq, np.float32)
    Wkv = np.asarray(Wkv, np.float32)
    bkv = np.asarray(bkv, np.float32)
    Wp = np.asarray(Wp, np.float32)
    bp = np.asarray(bp, np.float32)

    Wk = Wkv[:, :D]
    Wv = Wkv[:, D:]
    bv = bkv[D:]
    bpe = (bv @ Wp + bp).astype(NP_BF16).reshape(1, D)

    wq_h = _chunk_rows(Wq).astype(NP_BF16)
    wk_h = _chunk_rows(Wk).astype(NP_BF16)
    wv_h = _chunk_rows(Wv).astype(NP_BF16)
    # Wp rows head-major: wp4[d, h*256+m] = Wp[h*64+d, m]
    wp4 = np.ascontiguousarray(
        Wp.reshape(H, HD, D).transpose(1, 0, 2).reshape(HD, H * D)
    ).astype(NP_BF16)
    # bq packed: [128, H/2]; partition p=(t*64+d), col hp -> bq[(2hp+t)*64+d]
    bqp = np.ascontiguousarray(
        bq.reshape(H // 2, 2, HD).transpose(1, 2, 0).reshape(P, H // 2)
    )

    weights_map = {"wq": wq_h, "wk": wk_h, "wv": wv_h,
                   "wp4": wp4, "bqp": bqp, "bpe": bpe}

    def core_map(xr, cb):
        return {
            "xnat": _pack_rows(xr, lq // P),
            "xt": np.ascontiguousarray(xr.T.reshape(2, P, lq)
                                       .transpose(1, 0, 2).reshape(P, 2 * lq))
                  .astype(NP_BF16),
            "ctxt": np.ascontiguousarray(cb.T.reshape(2, P, lc)
                                         .transpose(1, 0, 2).reshape(P, 2 * lc))
                    .astype(NP_BF16),
            **weights_map,
        }

    in_maps = []
    for c in range(n_cores):
        b = c // (n_cores // B)
        s = c % (n_cores // B)
        in_maps.append(core_map(x[b, s * lq : (s + 1) * lq], context[b, :lc]))
    return in_maps


def _unpack_out(o, lq=LQ):
    nbq = lq // P
    return o.reshape(P, nbq, D).transpose(1, 0, 2).reshape(lq, D)


def run_sharded(inputs, trace=False, **kw):
    nc = build_program()
    in_maps = make_in_maps(**inputs)
    res = run_bass_kernel_spmd(nc, in_maps, list(range(N_CORES)), trace=trace,
                               **kw)
    out = np.empty((B, L, D), np.float32)
    per_b = N_CORES // B
    for c in range(N_CORES):
        b, s = c // per_b, c % per_b
        out[b, s * LQ : (s + 1) * LQ] = _unpack_out(res.results[c]["out"])
    attn_map = np.ones((B, int(L ** 0.5), int(L ** 0.5)), np.float32)
    return (out, attn_map), res


def kernel(**inputs):
    (out, attn_map), _ = run_sharded(inputs)
    return (out, attn_map)


# revision 15
# speedup vs baseline: 1.1268x; 1.1268x over previous
"""Trainium2 Bass kernel for CrossAttention_Light.

Problem: B=4, L=Lc=4096, D=256, H=4 heads of hd=64.
  q  = x @ Wq + bq                      -> [B,H,L,64]
  k,v = context @ Wkv + bkv             -> [B,H,Lc,64]
  P  = softmax(q k^T / 8)               -> [B,H,L,Lc]
  attn_map = mean_h sum_k P == 1.0 (exactly, softmax rows sum to 1)
  out = (P v) concat-heads @ Wp + bp + x

Sharding: 8 cores = 4 batches x 2 query-halves (2048 rows each). Each core
computes its full output rows end-to-end; no cross-core communication.

Math simplifications (exact):
  - k-bias drops out of softmax (per-row shift invariance).
  - v-bias: P @ (V+bv) = P@V + bv (rows of P sum to 1) -> folded into the
    projection bias: bp_eff = bv @ Wp + bp.
  - softmax without max-subtraction: scores = q.k/8 with |score| < ~1 for
    this weight scale (0.02), exp() is numerically safe.

Device algorithm per core (flash-style, scores never touch HBM):
  - all matmul operands bf16 (f32 PSUM accumulation); errors average over
    the 4096-term attention sums and the f32 residual dominates the output.
  - S^T layout: S^T[k,q] blocks from lhsT=K^T[64,128], rhs=Q^T[64,512].
    Head pairs occupy partition halves (rows 0-63 / 64-127) so the two
    S matmuls (K=64 each) run concurrently in the PE array.
  - exp on ACT (scale=1/8 folded in), one [128,1024] instr per k-block
    covering both heads.
  - AV: lhsT=[V|1] [128,65], rhs=expS^T [128,512] accumulating
    O^T[65,512] in PSUM; row 64 = softmax denominators.
  - normalize: reciprocal (DVE) + PE rank-1 broadcast + DVE multiply.
  - projection: per 128-row block, 4 head matmuls + rank-1 bias matmul,
    residual add (DVE), DMA out.
"""

import os
import sys

for _p in ("/opt/trn_rl_repo",):
    if _p not in sys.path:
        sys.path.append(_p)

import ml_dtypes
import numpy as np
from contextlib import ExitStack

NP_BF16 = ml_dtypes.bfloat16

import concourse.bass as bass
import concourse.mybir as mybir
import concourse.tile as tile
from concourse import bacc
from concourse.bass_utils import run_bass_kernel_spmd

F32 = mybir.dt.float32
BF16 = mybir.dt.bfloat16
AF = mybir.ActivationFunctionType

B, L, D, LC, H, HD = 4, 4096, 256, 4096, 4, 64
N_CORES = 8
LQ = L // 2          # q rows per core
P = 128
SB = 512             # q superblock
SCALE = HD ** -0.5


def build_program(lq=LQ, lc=LC, n_cores=N_CORES):
    """Build the SPMD Bass program. Returns nc."""
    nc = bacc.Bacc(
        "TRN2", target_bir_lowering=False, debug=False, num_devices=n_cores
    )
    nbq = lq // P        # q blocks
    nbc = lc // P        # k blocks
    nsb = lq // SB       # q superblocks
    dc = D // P          # contraction chunks (2)
    hp_n = H // 2        # head pairs

    def din(name, shape, dt=F32):
        return nc.dram_tensor(name, shape, dt, kind="ExternalInput").ap()

    x_d = din("xnat", [P, nbq * D])          # x rows, partition-packed
    xt_d = din("xt", [P, dc * lq], BF16)     # x^T chunks
    ct_d = din("ctxt", [P, dc * lc], BF16)   # context^T chunks
    wq_d = din("wq", [P, dc * D], BF16)      # Wq row-chunked
    wk_d = din("wk", [P, dc * D], BF16)
    wv_d = din("wv", [P, dc * D], BF16)
    wp_d = din("wp4", [HD, H * D], BF16)     # Wp head-major rows
    bq_d = din("bqp", [P, hp_n])             # q bias packed (t d) hp
    bp_d = din("bpe", [1, D], BF16)          # effective out bias
    out_d = nc.dram_tensor("out", [P, nbq * D], F32, kind="ExternalOutput").ap()

    with tile.TileContext(nc) as tc, ExitStack() as ctx:
        _kernel(ctx, tc, x_d, xt_d, ct_d, wq_d, wk_d, wv_d, wp_d, bq_d, bp_d,
                out_d, lq, lc)
    nc.compile()
    return nc


def _kernel(ctx, tc, x_d, xt_d, ct_d, wq_d, wk_d, wv_d, wp_d, bq_d, bp_d,
            out_d, lq, lc):
    nc = tc.nc
    nbq = lq // P
    nbc = lc // P
    nsb = lq // SB
    dc = D // P
    hp_n = H // 2

    persist = ctx.enter_context(tc.tile_pool(name="persist", bufs=1))

    # ---- persistent SBUF tiles ----
    x_sb = persist.tile([P, nbq, D], F32, tag="x")
    qT = [persist.tile([P, lq], BF16, tag=f"qT{i}", name=f"qT{i}")
          for i in range(hp_n)]
    kT = [persist.tile([P, lc], BF16, tag=f"kT{i}", name=f"kT{i}")
          for i in range(hp_n)]
    vones = persist.tile([P, nbc, H, HD + 1], BF16, tag="vones")
    nT = persist.tile([HD, H, lq], BF16, tag="nT")
    wq_sb = persist.tile([P, dc, D], BF16, tag="wq")
    wk_sb = persist.tile([P, dc, D], BF16, tag="wk")
    wv_sb = persist.tile([P, dc, D], BF16, tag="wv")
    wp_sb = persist.tile([HD, H, D], BF16, tag="wp")
    bq_sb = persist.tile([P, hp_n], F32, tag="bq")
    bp_sb = persist.tile([1, D], BF16, tag="bp")
    ones1 = persist.tile([1, P], BF16, tag="ones")
    ones1f = persist.tile([1, P], F32, tag="onesf")

    nc.sync.dma_start(x_sb[:].rearrange("p n d -> p (n d)"), x_d[:])
    nc.sync.dma_start(wq_sb[:].rearrange("p c d -> p (c d)"), wq_d[:])
    nc.sync.dma_start(wk_sb[:].rearrange("p c d -> p (c d)"), wk_d[:])
    nc.sync.dma_start(wv_sb[:].rearrange("p c d -> p (c d)"), wv_d[:])
    nc.sync.dma_start(wp_sb[:].rearrange("p h d -> p (h d)"), wp_d[:])
    nc.sync.dma_start(bq_sb[:], bq_d[:])
    nc.sync.dma_start(bp_sb[:], bp_d[:])
    nc.vector.memset(ones1[:], 1.0)
    nc.vector.memset(ones1f[:], 1.0)
    nc.vector.memset(vones[:, :, :, HD : HD + 1], 1.0)

    # ---- phase A: projections. Order K0, Q0 (unblock attention asap),
    # then V (first AV needs it ~30us in), then K1, Q1 (overlap attention).
    pa_in = ctx.enter_context(tc.tile_pool(name="phaseA_in", bufs=1))
    pq_pool = ctx.enter_context(tc.tile_pool(name="pq", bufs=2, space="PSUM"))
    xt_sb = pa_in.tile([P, dc, lq], BF16, tag="xt")
    ct_sb = pa_in.tile([P, dc, lc], BF16, tag="ct")
    nc.sync.dma_start(ct_sb[:].rearrange("p c q -> p (c q)"), ct_d[:])
    nc.sync.dma_start(xt_sb[:].rearrange("p c q -> p (c q)"), xt_d[:])

    def proj_qk(hp, w_sb, src, src_len, dst, bias):
        # dst[t*64+(0:64), nb*512:+512] = head (2hp+t) projection
        for nb in range(src_len // SB):
            pq = pq_pool.tile([P, SB], F32, tag="pq")
            for t in range(2):
                h = 2 * hp + t
                tp = (0, 64) if t == 1 else None
                for c in range(dc):
                    nc.tensor.matmul(
                        pq[t * HD : (t + 1) * HD, :],
                        w_sb[:, c, h * HD : (h + 1) * HD],
                        src[:, c, nb * SB : (nb + 1) * SB],
                        start=(c == 0), stop=(c == dc - 1),
                        tile_position=tp,
                    )
            sl = dst[:, nb * SB : (nb + 1) * SB]
            if bias is not None:
                nc.vector.tensor_scalar_add(sl, pq[:], bias)
            else:
                nc.vector.tensor_copy(sl, pq[:])

    def proj_v():
        for kb in range(nbc):
            pv = pq_pool.tile([P, D], F32, tag="pq", name="pv")
            for c in range(dc):
                nc.tensor.matmul(
                    pv[:],
                    ct_sb[:, c, kb * P : (kb + 1) * P],
                    wv_sb[:, c, :],
                    start=(c == 0), stop=(c == dc - 1),
                )
            nc.vector.tensor_copy(
                vones[:, kb, :, 0:HD],
                pv[:].rearrange("p (h e) -> p h e", e=HD),
            )

    proj_qk(0, wk_sb, ct_sb, lc, kT[0], None)
    proj_qk(0, wq_sb, xt_sb, lq, qT[0], bq_sb[:, 0:1])
    proj_v()
    proj_qk(1, wk_sb, ct_sb, lc, kT[1], None)
    proj_qk(1, wq_sb, xt_sb, lq, qT[1], bq_sb[:, 1:2])

    # ---- phase B: attention (sb-major so projection can follow each sb),
    # normalize moved off the PSUM critical path via an SBUF copy.
    ps_pool = ctx.enter_context(tc.tile_pool(name="ps", bufs=2, space="PSUM"))
    po_pool = ctx.enter_context(tc.tile_pool(name="po", bufs=2, space="PSUM"))
    es_pool = ctx.enter_context(tc.tile_pool(name="es", bufs=3))
    u_pool = ctx.enter_context(tc.tile_pool(name="u", bufs=6))
    rc_pool = ctx.enter_context(tc.tile_pool(name="rc", bufs=2))
    os_pool = ctx.enter_context(tc.tile_pool(name="os", bufs=3))

    def normalize(sb, hp, u):
        # recips first (DVE), then PE broadcasts, then muls: emitted mid
        # k-loop of the NEXT group so nothing here stalls PE or starves ACT
        rcs = []
        for t in range(2):
            rc = rc_pool.tile([1, SB], F32, tag="rc", name=f"rc{t}")
            nc.vector.reciprocal(rc[:], u[t][HD : HD + 1, :])
            rcs.append(rc)
        pbs = []
        for t in range(2):
            pb = pq_pool.tile([HD, SB], F32, tag="pq", name="pb")
            nc.tensor.matmul(pb[:], ones1f[0:1, 0:HD], rcs[t][:],
                             start=True, stop=True)
            pbs.append(pb)
        for t in range(2):
            nc.vector.tensor_mul(
                nT[:, 2 * hp + t, sb * SB : (sb + 1) * SB],
                u[t][0:HD, :], pbs[t][:],
            )

    def proj(sb):
        # projection + residual for this superblock's 4 row-blocks
        for j in range(sb * (SB // P), (sb + 1) * (SB // P)):
            pp = pq_pool.tile([P, D], F32, tag="pq", name="pp")
            for h in range(H):
                nc.tensor.matmul(
                    pp[:],
                    nT[:, h, j * P : (j + 1) * P],
                    wp_sb[:, h, :],
                    start=(h == 0), stop=False,
                )
            nc.tensor.matmul(pp[:], ones1[0:1, :], bp_sb[:],
                             start=False, stop=True)
            ost = os_pool.tile([P, D], F32, tag="os", name="ost")
            nc.vector.tensor_add(ost[:], pp[:], x_sb[:, j, :])
            nc.sync.dma_start(out_d[:, j * D : (j + 1) * D], ost[:])

    pending = []
    for sb in range(nsb):
        for hp in range(hp_n):
            po = [po_pool.tile([HD + 1, SB], F32, tag="po", name=f"po{t}")
                  for t in range(2)]
            for kb in range(nbc):
                # splice the previous group's deferred epilogue into the
                # middle of this k-loop
                if kb in (10, 20) and pending:
                    pending.pop(0)()
                ps = ps_pool.tile([P, 2 * SB], F32, tag="ps")
                for t in range(2):
                    nc.tensor.matmul(
                        ps[:, t * SB : (t + 1) * SB],
                        kT[hp][t * HD : (t + 1) * HD, kb * P : (kb + 1) * P],
                        qT[hp][t * HD : (t + 1) * HD, sb * SB : (sb + 1) * SB],
                        start=True, stop=True,
                    )
                es = es_pool.tile([P, 2 * SB], BF16, tag="es")
                nc.scalar.activation(es[:], ps[:], AF.Exp, scale=SCALE)
                for t in range(2):
                    nc.tensor.matmul(
                        po[t],
                        vones[:, kb, 2 * hp + t, :],
                        es[:, t * SB : (t + 1) * SB],
                        start=(kb == 0), stop=(kb == nbc - 1),
                    )
            # quick PSUM->SBUF copies release po for the next (sb,hp)
            u = [u_pool.tile([HD + 1, SB], F32, tag="u", name=f"u{t}")
                 for t in range(2)]
            for t in range(2):
                nc.vector.tensor_copy(u[t][:], po[t])
            pending.append(lambda sb=sb, hp=hp, u=u: normalize(sb, hp, u))
            if hp == hp_n - 1:
                pending.append(lambda sb=sb: proj(sb))
    for f in pending:
        f()


def _pack_rows(a, nb):
    """[nb*128, d] -> [128, nb*d] partition-packed layout."""
    n, d = a.shape
    return np.ascontiguousarray(
        a.reshape(nb, P, d).transpose(1, 0, 2).reshape(P, nb * d)
    )


def _chunk_rows(w):
    """[256, d] -> [128, 2*d]: row chunks side by side."""
    r, d = w.shape
    c = r // P
    return np.ascontiguousarray(
        w.reshape(c, P, d).transpose(1, 0, 2).reshape(P, c * d)
    )


def make_in_maps(x, context, Wq, bq, Wkv, bkv, Wp, bp, lq=LQ, lc=LC,
                 n_cores=N_CORES):
    x = np.asarray(x, np.float32)
    context = np.asarray(context, np.float32)
    Wq = np.asarray(Wq, np.float32)
    bq = np.asarray(bq, np.float32)
    Wkv = np.asarray(Wkv, np.float32)
    bkv = np.asarray(bkv, np.float32)
    Wp = np.asarray(Wp, np.float32)
    bp = np.asarray(bp, np.float32)

    Wk = Wkv[:, :D]
    Wv = Wkv[:, D:]
    bv = bkv[D:]
    bpe = (bv @ Wp + bp).astype(NP_BF16).reshape(1, D)

    wq_h = _chunk_rows(Wq).astype(NP_BF16)
    wk_h = _chunk_rows(Wk).astype(NP_BF16)
    wv_h = _chunk_rows(Wv).astype(NP_BF16)
    # Wp rows head-major: wp4[d, h*256+m] = Wp[h*64+d, m]
    wp4 = np.ascontiguousarray(
        Wp.reshape(H, HD, D).transpose(1, 0, 2).reshape(HD, H * D)
    ).astype(NP_BF16)
    # bq packed: [128, H/2]; partition p=(t*64+d), col hp -> bq[(2hp+t)*64+d]
    bqp = np.ascontiguousarray(
        bq.reshape(H // 2, 2, HD).transpose(1, 2, 0).reshape(P, H // 2)
    )

    weights_map = {"wq": wq_h, "wk": wk_h, "wv": wv_h,
                   "wp4": wp4, "bqp": bqp, "bpe": bpe}

    def core_map(xr, cb):
        return {
            "xnat": _pack_rows(xr, lq // P),
            "xt": np.ascontiguousarray(xr.T.reshape(2, P, lq)
                                       .transpose(1, 0, 2).reshape(P, 2 * lq))
                  .astype(NP_BF16),
            "ctxt": np.ascontiguousarray(cb.T.reshape(2, P, lc)
                                         .transpose(1, 0, 2).reshape(P, 2 * lc))
                    .astype(NP_BF16),
            **weights_map,
        }

    in_maps = []
    for c in range(n_cores):
        b = c // (n_cores // B)
        s = c % (n_cores // B)
        in_maps.append(core_map(x[b, s * lq : (s + 1) * lq], context[b, :lc]))
    return in_maps


def _unpack_out(o, lq=LQ):
    nbq = lq // P
    return o.reshape(P, nbq, D).transpose(1, 0, 2).reshape(lq, D)


def run_sharded(inputs, trace=False, **kw):
    nc = build_program()
    in_maps = make_in_maps(**inputs)
    res = run_bass_kernel_spmd(nc, in_maps, list(range(N_CORES)), trace=trace,
                               **kw)
    out = np.empty((B, L, D), np.float32)
    per_b = N_CORES // B
    for c in range(N_CORES):
        b, s = c // per_b, c % per_b
        out[b, s * LQ : (s + 1) * LQ] = _unpack_out(res.results[c]["out"])
    attn_map = np.ones((B, int(L ** 0.5), int(L ** 0.5)), np.float32)
    return (out, attn_map), res


def kernel(**inputs):
    (out, attn_map), _ = run_sharded(inputs)
    return (out, attn_map)
